# revision 2
# baseline (speedup 1.0000x reference)
"""Trainium2 kernel for nn_Circuit_28123445854302.

24-wire statevector circuit (depth-4 brickwork, 46 two-qubit gates) applied to
a product state.  Strategy:

The statevector is sharded over its 3 leading wire axes across the 8 cores
(state-index sharding, as hinted).  Rather than streaming the 64 MB state
through every gate, we exploit the circuit's 1-D locality: across the middle
wire cut (12|12) only the gates that straddle the cut can raise the Schmidt
rank, so the final state factors EXACTLY as

    psi[left, right] = sum_r A[r, left] * B[r, right]        (rank R, tiny)

For the brickwork circuit R == 16.  A and B (R x 4096) are computed exactly on
the host in float64 with negligible cost (all tensors are O(R * 2^12)); every
element of the 2^24 statevector is then produced ON DEVICE by a K=R matmul:

    core c:  out[512, 4096] = A[:, c*512:(c+1)*512].T @ B        (rows = left
             indices with leading-3-wire bits == c, i.e. the core's shard)

Each core writes its contiguous 8 MB shard of the output; the host gather is a
plain concatenate.  This puts the kernel at the memory roofline: the 64 MB
output write is the unavoidable traffic and the matmul streams at the same
rate the DMA drains.

If a (hypothetical) non-local gate list makes the cut rank explode, we fall
back to an exact dense numpy simulation (same semantics as the reference).
"""

import numpy as np

_N_WIRES = 24
_CUT = 12
_HALF = 1 << _CUT          # 4096
_N_CORES = 8
_ROWS_PER_CORE = _HALF // _N_CORES   # 512
_MAX_RANK = 512


# ----------------------------------------------------------------------------
# Host-side exact middle-cut factorization (all tiny tensors, float64)
# ----------------------------------------------------------------------------

def _apply_2q(M, g, w0, w1, nloc):
    """Apply gate g[i0,o0,i1,o1] on local wires w0,w1 of every row of
    M (R, 2**nloc).  Matches reference: tensordot + moveaxis."""
    R = M.shape[0]
    T = M.reshape((R,) + (2,) * nloc)
    src = [4] + [0 if k == w0 else (2 if k == w1 else 5 + k) for k in range(nloc)]
    dst = [4] + [1 if k == w0 else (3 if k == w1 else 5 + k) for k in range(nloc)]
    return np.einsum(g, [0, 1, 2, 3], T, src, dst).reshape(R, -1)


def _apply_1q(M, P, w, nloc):
    """Apply P[i,o] on local wire w of every row of M (R, 2**nloc)."""
    R = M.shape[0]
    T = M.reshape((R,) + (2,) * nloc)
    src = [4] + [0 if t == w else 5 + t for t in range(nloc)]
    dst = [4] + [1 if t == w else 5 + t for t in range(nloc)]
    return np.einsum(P, [0, 1], T, src, dst).reshape(R, -1)


def _build_factors(states, gates, gate_wires):
    """psi = A.T @ B with A, B (R, 4096) float64, or None if rank > _MAX_RANK."""
    states = np.asarray(states, dtype=np.float64)
    gates = np.asarray(gates, dtype=np.float64)
    wires = np.asarray(gate_wires)
    NR = _N_WIRES - _CUT

    def outer(lo, hi):
        v = states[lo]
        for w in range(lo + 1, hi):
            v = np.kron(v, states[w])
        return v

    A = outer(0, _CUT)[None, :].copy()
    B = outer(_CUT, _N_WIRES)[None, :].copy()

    for gi in range(gates.shape[0]):
        w0, w1 = int(wires[gi, 0]), int(wires[gi, 1])
        g = gates[gi]
        if w0 == w1:
            return None  # ill-defined for the reference too; bail out
        if w0 > w1:
            g = np.transpose(g, (2, 3, 0, 1))
            w0, w1 = w1, w0
        if w1 < _CUT:
            A = _apply_2q(A, g, w0, w1, _CUT)
        elif w0 >= _CUT:
            B = _apply_2q(B, g, w0 - _CUT, w1 - _CUT, NR)
        else:
            # Gate straddles the cut: operator-Schmidt split (rank <= 4).
            M4 = g.reshape(4, 4)  # rows (i0,o0) act left, cols (i1,o1) act right
            U, s, Vt = np.linalg.svd(M4)
            rank = max(1, int((s > s[0] * 1e-14).sum()))
            newA, newB = [], []
            for k in range(rank):
                P = (U[:, k] * s[k]).reshape(2, 2)
                Q = Vt[k].reshape(2, 2)
                newA.append(_apply_1q(A, P, w0, _CUT))
                newB.append(_apply_1q(B, Q, w1 - _CUT, NR))
            A = np.concatenate(newA, 0)
            B = np.concatenate(newB, 0)
            # Exact recompression (drops only numerically-zero directions).
            if A.shape[0] > 4:
                qa, ra = np.linalg.qr(A.T)
                qb, rb = np.linalg.qr(B.T)
                u, sv, vt = np.linalg.svd(ra @ rb.T)
                keep = max(1, int((sv > (sv[0] if sv.size else 1.0) * 1e-13).sum()))
                A = (qa @ (u[:, :keep] * sv[:keep])).T
                B = vt[:keep] @ qb.T
            if A.shape[0] > _MAX_RANK:
                return None
    return A, B


# ----------------------------------------------------------------------------
# Dense fallback (exact reference semantics in numpy) — only used if the gate
# list is so non-local that the middle-cut rank explodes.
# ----------------------------------------------------------------------------

def _dense_fallback(states, gates, gate_wires):
    states = np.asarray(states, dtype=np.float32)
    gates = np.asarray(gates, dtype=np.float32)
    wires = np.asarray(gate_wires)
    psi = states[0]
    for w in range(1, _N_WIRES):
        psi = np.multiply.outer(psi, states[w])
    for g in range(gates.shape[0]):
        w0, w1 = int(wires[g, 0]), int(wires[g, 1])
        psi = np.tensordot(gates[g], psi, axes=[[0, 2], [w0, w1]])
        psi = np.moveaxis(psi, (0, 1), (w0, w1))
    return psi


# ----------------------------------------------------------------------------
# Device kernel: out[512, 4096] = lhsT.T @ rhs  per core
# ----------------------------------------------------------------------------

_COMPILED = {}


def _get_nc(R):
    """Build (and cache) the Bass module for contraction rank R."""
    if R in _COMPILED:
        return _COMPILED[R]
    import concourse.bass as bass
    import concourse.tile as tile
    from concourse import bacc, mybir

    nc = bacc.Bacc(
        "TRN2",
        target_bir_lowering=False,
        debug=False,
        enable_asserts=False,
        num_devices=_N_CORES,
    )
    dt = mybir.dt.float32
    lhsT = nc.dram_tensor("lhsT", [R, _ROWS_PER_CORE], dt, kind="ExternalInput").ap()
    rhs = nc.dram_tensor("rhs", [R, _HALF], dt, kind="ExternalInput").ap()
    out = nc.dram_tensor("out", [_ROWS_PER_CORE, _HALF], dt, kind="ExternalOutput").ap()

    n_mchunk = _ROWS_PER_CORE // 128          # 4
    n_nchunk = _HALF // 512                   # 8
    n_kchunk = (R + 127) // 128               # 1 for brickwork (R=16)

    with tile.TileContext(nc) as tc:
        with (
            tc.tile_pool(name="const", bufs=1) as cpool,
            tc.tile_pool(name="psum", bufs=8, space=bass.MemorySpace.PSUM) as ppool,
            tc.tile_pool(name="outs", bufs=2) as opool,
        ):
            lhsT_sb = cpool.tile([R, _ROWS_PER_CORE], dt)
            nc.sync.dma_start(lhsT_sb[:], lhsT[:])
            rhs_sb = cpool.tile([R, _HALF], dt)
            nc.sync.dma_start(rhs_sb[:], rhs[:])

            for m in range(n_mchunk):
                ot = opool.tile([128, _HALF], dt)
                for n in range(n_nchunk):
                    ps = ppool.tile([128, 512], dt)
                    for kc in range(n_kchunk):
                        k0, k1 = kc * 128, min(R, (kc + 1) * 128)
                        nc.tensor.matmul(
                            ps[:],
                            lhsT_sb[k0:k1, m * 128:(m + 1) * 128],
                            rhs_sb[k0:k1, n * 512:(n + 1) * 512],
                            start=(kc == 0),
                            stop=(kc == n_kchunk - 1),
                        )
                    # Split PSUM->SBUF copies across both elementwise engines.
                    if n % 2 == 0:
                        nc.vector.tensor_copy(ot[:, n * 512:(n + 1) * 512], ps[:])
                    else:
                        nc.scalar.copy(ot[:, n * 512:(n + 1) * 512], ps[:])
                nc.sync.dma_start(out[m * 128:(m + 1) * 128, :], ot[:])
    nc.compile()
    _COMPILED[R] = nc
    return nc


def _make_in_maps(A, B):
    rhs_np = np.ascontiguousarray(B, dtype=np.float32)
    in_maps = []
    for c in range(_N_CORES):
        in_maps.append({
            "lhsT": np.ascontiguousarray(
                A[:, c * _ROWS_PER_CORE:(c + 1) * _ROWS_PER_CORE], dtype=np.float32
            ),
            "rhs": rhs_np,
        })
    return in_maps


def _run_device(A, B, trace=False):
    """A, B: (R, 4096) float32.  Returns (psi_flat float32 (2^24,), results)."""
    from concourse.bass_utils import run_bass_kernel_spmd

    R = A.shape[0]
    nc = _get_nc(R)
    in_maps = _make_in_maps(A, B)
    res = run_bass_kernel_spmd(
        nc, in_maps, core_ids=list(range(_N_CORES)), trace=trace
    )
    flat = np.concatenate([r["out"].reshape(-1) for r in res.results])
    return flat, res


def kernel(states, gates, gate_wires):
    fact = _build_factors(states, gates, gate_wires)
    if fact is None:
        return _dense_fallback(states, gates, gate_wires)
    A, B = fact
    flat, _ = _run_device(A.astype(np.float32), B.astype(np.float32))
    return flat.reshape((2,) * _N_WIRES)


# revision 7
# speedup vs baseline: 1.0991x; 1.0991x over previous
"""Trainium2 kernel for nn_Circuit_28123445854302.

24-wire statevector circuit (depth-4 brickwork, 46 two-qubit gates) applied to
a product state.  Strategy:

The statevector is sharded over its 3 leading wire axes across the 8 cores
(state-index sharding, as hinted).  Rather than streaming the 64 MB state
through every gate, we exploit the circuit's 1-D locality: across the middle
wire cut (12|12) only the gates that straddle the cut can raise the Schmidt
rank, so the final state factors EXACTLY as

    psi[left, right] = sum_r A[r, left] * B[r, right]        (rank R, tiny)

For the brickwork circuit R == 16.  A and B (R x 4096) are computed exactly on
the host in float64 with negligible cost (all tensors are O(R * 2^12)); every
element of the 2^24 statevector is then produced ON DEVICE by a K=R matmul:

    core c:  out[512, 4096] = A[:, c*512:(c+1)*512].T @ B        (rows = left
             indices with leading-3-wire bits == c, i.e. the core's shard)

Each core writes its contiguous 8 MB shard of the output; the host gather is a
plain concatenate.  This puts the kernel at the memory roofline: the 64 MB
output write is the unavoidable traffic and the matmul streams at the same
rate the DMA drains.

If a (hypothetical) non-local gate list makes the cut rank explode, we fall
back to an exact dense numpy simulation (same semantics as the reference).
"""

import numpy as np

_N_WIRES = 24
_CUT = 12
_HALF = 1 << _CUT          # 4096
_N_CORES = 8
_ROWS_PER_CORE = _HALF // _N_CORES   # 512
_MAX_RANK = 512


# ----------------------------------------------------------------------------
# Host-side exact middle-cut factorization (all tiny tensors, float64)
# ----------------------------------------------------------------------------

def _apply_2q(M, g, w0, w1, nloc):
    """Apply gate g[i0,o0,i1,o1] on local wires w0,w1 of every row of
    M (R, 2**nloc).  Matches reference: tensordot + moveaxis."""
    R = M.shape[0]
    T = M.reshape((R,) + (2,) * nloc)
    src = [4] + [0 if k == w0 else (2 if k == w1 else 5 + k) for k in range(nloc)]
    dst = [4] + [1 if k == w0 else (3 if k == w1 else 5 + k) for k in range(nloc)]
    return np.einsum(g, [0, 1, 2, 3], T, src, dst).reshape(R, -1)


def _apply_1q(M, P, w, nloc):
    """Apply P[i,o] on local wire w of every row of M (R, 2**nloc)."""
    R = M.shape[0]
    T = M.reshape((R,) + (2,) * nloc)
    src = [4] + [0 if t == w else 5 + t for t in range(nloc)]
    dst = [4] + [1 if t == w else 5 + t for t in range(nloc)]
    return np.einsum(P, [0, 1], T, src, dst).reshape(R, -1)


def _build_factors(states, gates, gate_wires):
    """psi = A.T @ B with A, B (R, 4096) float64, or None if rank > _MAX_RANK."""
    states = np.asarray(states, dtype=np.float64)
    gates = np.asarray(gates, dtype=np.float64)
    wires = np.asarray(gate_wires)
    NR = _N_WIRES - _CUT

    def outer(lo, hi):
        v = states[lo]
        for w in range(lo + 1, hi):
            v = np.kron(v, states[w])
        return v

    A = outer(0, _CUT)[None, :].copy()
    B = outer(_CUT, _N_WIRES)[None, :].copy()

    for gi in range(gates.shape[0]):
        w0, w1 = int(wires[gi, 0]), int(wires[gi, 1])
        g = gates[gi]
        if w0 == w1:
            return None  # ill-defined for the reference too; bail out
        if w0 > w1:
            g = np.transpose(g, (2, 3, 0, 1))
            w0, w1 = w1, w0
        if w1 < _CUT:
            A = _apply_2q(A, g, w0, w1, _CUT)
        elif w0 >= _CUT:
            B = _apply_2q(B, g, w0 - _CUT, w1 - _CUT, NR)
        else:
            # Gate straddles the cut: operator-Schmidt split (rank <= 4).
            M4 = g.reshape(4, 4)  # rows (i0,o0) act left, cols (i1,o1) act right
            U, s, Vt = np.linalg.svd(M4)
            rank = max(1, int((s > s[0] * 1e-14).sum()))
            newA, newB = [], []
            for k in range(rank):
                P = (U[:, k] * s[k]).reshape(2, 2)
                Q = Vt[k].reshape(2, 2)
                newA.append(_apply_1q(A, P, w0, _CUT))
                newB.append(_apply_1q(B, Q, w1 - _CUT, NR))
            A = np.concatenate(newA, 0)
            B = np.concatenate(newB, 0)
            # Exact recompression (drops only numerically-zero directions).
            if A.shape[0] > 4:
                qa, ra = np.linalg.qr(A.T)
                qb, rb = np.linalg.qr(B.T)
                u, sv, vt = np.linalg.svd(ra @ rb.T)
                keep = max(1, int((sv > (sv[0] if sv.size else 1.0) * 1e-13).sum()))
                A = (qa @ (u[:, :keep] * sv[:keep])).T
                B = vt[:keep] @ qb.T
            if A.shape[0] > _MAX_RANK:
                return None
    return A, B


# ----------------------------------------------------------------------------
# Dense fallback (exact reference semantics in numpy) — only used if the gate
# list is so non-local that the middle-cut rank explodes.
# ----------------------------------------------------------------------------

def _dense_fallback(states, gates, gate_wires):
    states = np.asarray(states, dtype=np.float32)
    gates = np.asarray(gates, dtype=np.float32)
    wires = np.asarray(gate_wires)
    psi = states[0]
    for w in range(1, _N_WIRES):
        psi = np.multiply.outer(psi, states[w])
    for g in range(gates.shape[0]):
        w0, w1 = int(wires[g, 0]), int(wires[g, 1])
        psi = np.tensordot(gates[g], psi, axes=[[0, 2], [w0, w1]])
        psi = np.moveaxis(psi, (0, 1), (w0, w1))
    return psi


# ----------------------------------------------------------------------------
# Device kernel: out[512, 4096] = lhsT.T @ rhs  per core
# ----------------------------------------------------------------------------

_COMPILED = {}

# Best-known device configuration (updated as HW experiments come in).
_CFG = dict(mm_dtype="float32r", psum_cols=512, act_share=2, repeat=1, out_split=8, in_split=8)


def _build_nc(R, mm_dtype="float32r", psum_cols=1024, act_share=0, repeat=1,
              out_split=1, in_split=1):
    """Build the Bass module for contraction rank R.

    mm_dtype:  dtype used for the matmul operands ("float32", "float32r").
               float32r streams the moving operand at 1 cycle/row (vs 4 for
               float32) when N >= 256.
    psum_cols: width of each PSUM tile (multiple of 512; one matmul per
               512-wide bank slice, one PSUM->SBUF copy per tile).
    act_share: of every 4 PSUM->SBUF copies, how many go to ScalarE
               (the rest go to VectorE).
    repeat:    unrolled repetitions of the whole kernel (for differential
               timing; output is rewritten identically each time).
    """
    import concourse.bass as bass
    import concourse.tile as tile
    from concourse import bacc, mybir

    nc = bacc.Bacc(
        "TRN2",
        target_bir_lowering=False,
        debug=False,
        enable_asserts=False,
        num_devices=_N_CORES,
    )
    dt32 = mybir.dt.float32
    mdt = getattr(mybir.dt, mm_dtype)
    lhsT = nc.dram_tensor("lhsT", [R, _ROWS_PER_CORE], mdt, kind="ExternalInput").ap()
    rhs = nc.dram_tensor("rhs", [R, _HALF], mdt, kind="ExternalInput").ap()
    out = nc.dram_tensor("out", [_ROWS_PER_CORE, _HALF], dt32, kind="ExternalOutput").ap()

    n_mchunk = _ROWS_PER_CORE // 128          # 4
    n_kchunk = (R + 127) // 128               # 1 for brickwork (R=16)
    mm_per_ps = psum_cols // 512
    n_pschunk = _HALF // psum_cols

    with tile.TileContext(nc) as tc:
        with (
            tc.tile_pool(name="const", bufs=1) as cpool,
            tc.tile_pool(name="psum", bufs=8 // mm_per_ps, space=bass.MemorySpace.PSUM) as ppool,
            tc.tile_pool(name="outs", bufs=2) as opool,
        ):
            lhsT_sb = cpool.tile([R, _ROWS_PER_CORE], mdt)
            nc.sync.dma_start(lhsT_sb[:], lhsT[:])
            rhs_sb = cpool.tile([R, _HALF], mdt)
            for s in range(in_split):
                w = _HALF // in_split
                nc.sync.dma_start(rhs_sb[:, s * w:(s + 1) * w],
                                  rhs[:, s * w:(s + 1) * w])

            copy_i = 0
            for _rep in range(repeat):
                for m in range(n_mchunk):
                    ot = opool.tile([128, _HALF], dt32)
                    for pc in range(n_pschunk):
                        ps = ppool.tile([128, psum_cols], dt32)
                        for j in range(mm_per_ps):
                            n0 = pc * psum_cols + j * 512
                            for kc in range(n_kchunk):
                                k0, k1 = kc * 128, min(R, (kc + 1) * 128)
                                nc.tensor.matmul(
                                    ps[:, j * 512:(j + 1) * 512],
                                    lhsT_sb[k0:k1, m * 128:(m + 1) * 128],
                                    rhs_sb[k0:k1, n0:n0 + 512],
                                    start=(kc == 0),
                                    stop=(kc == n_kchunk - 1),
                                )
                        dst = ot[:, pc * psum_cols:(pc + 1) * psum_cols]
                        if copy_i % 4 < act_share:
                            nc.scalar.copy(dst, ps[:])
                        else:
                            nc.vector.tensor_copy(dst, ps[:])
                        copy_i += 1
                        if out_split > 1:
                            # stream this stripe to HBM as soon as it's copied
                            per = n_pschunk // out_split
                            if (pc + 1) % per == 0:
                                c0 = (pc + 1 - per) * psum_cols
                                c1 = (pc + 1) * psum_cols
                                nc.sync.dma_start(
                                    out[m * 128:(m + 1) * 128, c0:c1],
                                    ot[:, c0:c1])
                    if out_split == 1:
                        nc.sync.dma_start(out[m * 128:(m + 1) * 128, :], ot[:])
    nc.compile()
    return nc


def _get_nc(R):
    """Build (and cache) the production Bass module for contraction rank R."""
    if R in _COMPILED:
        return _COMPILED[R]
    nc = _build_nc(R, **_CFG)
    _COMPILED[R] = nc
    return nc


def _make_in_maps(A, B):
    rhs_np = np.ascontiguousarray(B, dtype=np.float32)
    in_maps = []
    for c in range(_N_CORES):
        in_maps.append({
            "lhsT": np.ascontiguousarray(
                A[:, c * _ROWS_PER_CORE:(c + 1) * _ROWS_PER_CORE], dtype=np.float32
            ),
            "rhs": rhs_np,
        })
    return in_maps


def _run_device(A, B, trace=False):
    """A, B: (R, 4096) float32.  Returns (psi_flat float32 (2^24,), results)."""
    from concourse.bass_utils import run_bass_kernel_spmd

    R = A.shape[0]
    nc = _get_nc(R)
    in_maps = _make_in_maps(A, B)
    res = run_bass_kernel_spmd(
        nc, in_maps, core_ids=list(range(_N_CORES)), trace=trace
    )
    flat = np.concatenate([r["out"].reshape(-1) for r in res.results])
    return flat, res


def kernel(states, gates, gate_wires):
    fact = _build_factors(states, gates, gate_wires)
    if fact is None:
        return _dense_fallback(states, gates, gate_wires)
    A, B = fact
    flat, _ = _run_device(A.astype(np.float32), B.astype(np.float32))
    return flat.reshape((2,) * _N_WIRES)


# revision 10
# speedup vs baseline: 3052.7351x; 2777.4688x over previous
"""Trainium2 kernel for nn_Circuit_28123445854302.

24-wire statevector circuit (depth-4 brickwork, 46 two-qubit gates) applied to
a product state.  Strategy:

The statevector is sharded over its 3 leading wire axes across the 8 cores
(state-index sharding, as hinted).  Rather than streaming the 64 MB state
through every gate, we exploit the circuit's 1-D locality: across the middle
wire cut (12|12) only the gates that straddle the cut can raise the Schmidt
rank, so the final state factors EXACTLY as

    psi[left, right] = sum_r A[r, left] * B[r, right]        (rank R, tiny)

For the brickwork circuit R == 16.  A and B (R x 4096) are computed exactly on
the host in float64 with negligible cost (all tensors are O(R * 2^12)); every
element of the 2^24 statevector is then produced ON DEVICE by a K=R matmul:

    core c:  out[512, 4096] = A[:, c*512:(c+1)*512].T @ B        (rows = left
             indices with leading-3-wire bits == c, i.e. the core's shard)

Each core writes its contiguous 8 MB shard of the output; the host gather is a
plain concatenate.  This puts the kernel at the memory roofline: the 64 MB
output write is the unavoidable traffic and the matmul streams at the same
rate the DMA drains.

If a (hypothetical) non-local gate list makes the cut rank explode, we fall
back to an exact dense numpy simulation (same semantics as the reference).
"""

import numpy as np

_N_WIRES = 24
_CUT = 12
_HALF = 1 << _CUT          # 4096
_N_CORES = 8
_ROWS_PER_CORE = _HALF // _N_CORES   # 512
_MAX_RANK = 512


# ----------------------------------------------------------------------------
# Host-side exact middle-cut factorization (all tiny tensors, float64)
# ----------------------------------------------------------------------------

def _apply_2q(M, g, w0, w1, nloc):
    """Apply gate g[i0,o0,i1,o1] on local wires w0,w1 of every row of
    M (R, 2**nloc).  Matches reference: tensordot + moveaxis."""
    R = M.shape[0]
    T = M.reshape((R,) + (2,) * nloc)
    src = [4] + [0 if k == w0 else (2 if k == w1 else 5 + k) for k in range(nloc)]
    dst = [4] + [1 if k == w0 else (3 if k == w1 else 5 + k) for k in range(nloc)]
    return np.einsum(g, [0, 1, 2, 3], T, src, dst).reshape(R, -1)


def _apply_1q(M, P, w, nloc):
    """Apply P[i,o] on local wire w of every row of M (R, 2**nloc)."""
    R = M.shape[0]
    T = M.reshape((R,) + (2,) * nloc)
    src = [4] + [0 if t == w else 5 + t for t in range(nloc)]
    dst = [4] + [1 if t == w else 5 + t for t in range(nloc)]
    return np.einsum(P, [0, 1], T, src, dst).reshape(R, -1)


def _build_factors(states, gates, gate_wires):
    """psi = A.T @ B with A, B (R, 4096) float64, or None if rank > _MAX_RANK."""
    states = np.asarray(states, dtype=np.float64)
    gates = np.asarray(gates, dtype=np.float64)
    wires = np.asarray(gate_wires)
    NR = _N_WIRES - _CUT

    def outer(lo, hi):
        v = states[lo]
        for w in range(lo + 1, hi):
            v = np.kron(v, states[w])
        return v

    A = outer(0, _CUT)[None, :].copy()
    B = outer(_CUT, _N_WIRES)[None, :].copy()

    for gi in range(gates.shape[0]):
        w0, w1 = int(wires[gi, 0]), int(wires[gi, 1])
        g = gates[gi]
        if w0 == w1:
            return None  # ill-defined for the reference too; bail out
        if w0 > w1:
            g = np.transpose(g, (2, 3, 0, 1))
            w0, w1 = w1, w0
        if w1 < _CUT:
            A = _apply_2q(A, g, w0, w1, _CUT)
        elif w0 >= _CUT:
            B = _apply_2q(B, g, w0 - _CUT, w1 - _CUT, NR)
        else:
            # Gate straddles the cut: operator-Schmidt split (rank <= 4).
            M4 = g.reshape(4, 4)  # rows (i0,o0) act left, cols (i1,o1) act right
            U, s, Vt = np.linalg.svd(M4)
            rank = max(1, int((s > s[0] * 1e-14).sum()))
            newA, newB = [], []
            for k in range(rank):
                P = (U[:, k] * s[k]).reshape(2, 2)
                Q = Vt[k].reshape(2, 2)
                newA.append(_apply_1q(A, P, w0, _CUT))
                newB.append(_apply_1q(B, Q, w1 - _CUT, NR))
            A = np.concatenate(newA, 0)
            B = np.concatenate(newB, 0)
            # Exact recompression (drops only numerically-zero directions).
            if A.shape[0] > 4:
                qa, ra = np.linalg.qr(A.T)
                qb, rb = np.linalg.qr(B.T)
                u, sv, vt = np.linalg.svd(ra @ rb.T)
                keep = max(1, int((sv > (sv[0] if sv.size else 1.0) * 1e-13).sum()))
                A = (qa @ (u[:, :keep] * sv[:keep])).T
                B = vt[:keep] @ qb.T
            if A.shape[0] > _MAX_RANK:
                return None
    return A, B


# ----------------------------------------------------------------------------
# Dense fallback (exact reference semantics in numpy) — only used if the gate
# list is so non-local that the middle-cut rank explodes.
# ----------------------------------------------------------------------------

def _dense_fallback(states, gates, gate_wires):
    states = np.asarray(states, dtype=np.float32)
    gates = np.asarray(gates, dtype=np.float32)
    wires = np.asarray(gate_wires)
    psi = states[0]
    for w in range(1, _N_WIRES):
        psi = np.multiply.outer(psi, states[w])
    for g in range(gates.shape[0]):
        w0, w1 = int(wires[g, 0]), int(wires[g, 1])
        psi = np.tensordot(gates[g], psi, axes=[[0, 2], [w0, w1]])
        psi = np.moveaxis(psi, (0, 1), (w0, w1))
    return psi


# ----------------------------------------------------------------------------
# Device kernel: out[512, 4096] = lhsT.T @ rhs  per core
# ----------------------------------------------------------------------------

_COMPILED = {}

# Best-known device configuration (updated as HW experiments come in).
# bfloat16 + "split3" packing: each f32 factor x is split as hi=bf16(x),
# lo=bf16(x-hi); the rank-R contraction is widened to 3R rows
#   lhsT' = [A_hi; A_lo; A_hi],  rhs' = [B_hi; B_hi; B_lo]
# so one K=3R bf16 matmul (1 cycle/row, full PE rate) accumulates
# hi*hi + lo*hi + hi*lo in fp32 PSUM — fp32-grade accuracy (only the
# ~2^-18 lo*lo term is dropped) at float32r speed.
_CFG = dict(mm_dtype="bfloat16", psum_cols=512, act_share=2, repeat=1, out_split=8, in_split=8)
_SPLIT3 = True


def _build_nc(R, mm_dtype="float32r", psum_cols=1024, act_share=0, repeat=1,
              out_split=1, in_split=1):
    """Build the Bass module for contraction rank R.

    mm_dtype:  dtype used for the matmul operands ("float32", "float32r").
               float32r streams the moving operand at 1 cycle/row (vs 4 for
               float32) when N >= 256.
    psum_cols: width of each PSUM tile (multiple of 512; one matmul per
               512-wide bank slice, one PSUM->SBUF copy per tile).
    act_share: of every 4 PSUM->SBUF copies, how many go to ScalarE
               (the rest go to VectorE).
    repeat:    unrolled repetitions of the whole kernel (for differential
               timing; output is rewritten identically each time).
    """
    import concourse.bass as bass
    import concourse.tile as tile
    from concourse import bacc, mybir

    nc = bacc.Bacc(
        "TRN2",
        target_bir_lowering=False,
        debug=False,
        enable_asserts=False,
        num_devices=_N_CORES,
    )
    dt32 = mybir.dt.float32
    mdt = getattr(mybir.dt, mm_dtype)
    lhsT = nc.dram_tensor("lhsT", [R, _ROWS_PER_CORE], mdt, kind="ExternalInput").ap()
    rhs = nc.dram_tensor("rhs", [R, _HALF], mdt, kind="ExternalInput").ap()
    out = nc.dram_tensor("out", [_ROWS_PER_CORE, _HALF], dt32, kind="ExternalOutput").ap()

    n_mchunk = _ROWS_PER_CORE // 128          # 4
    n_kchunk = (R + 127) // 128               # 1 for brickwork (R=16)
    mm_per_ps = psum_cols // 512
    n_pschunk = _HALF // psum_cols

    with tile.TileContext(nc) as tc:
        with (
            tc.tile_pool(name="const", bufs=1) as cpool,
            tc.tile_pool(name="psum", bufs=8 // mm_per_ps, space=bass.MemorySpace.PSUM) as ppool,
            tc.tile_pool(name="outs", bufs=2) as opool,
        ):
            lhsT_sb = cpool.tile([R, _ROWS_PER_CORE], mdt)
            nc.sync.dma_start(lhsT_sb[:], lhsT[:])
            rhs_sb = cpool.tile([R, _HALF], mdt)
            for s in range(in_split):
                w = _HALF // in_split
                nc.sync.dma_start(rhs_sb[:, s * w:(s + 1) * w],
                                  rhs[:, s * w:(s + 1) * w])

            copy_i = 0
            for _rep in range(repeat):
                for m in range(n_mchunk):
                    ot = opool.tile([128, _HALF], dt32)
                    for pc in range(n_pschunk):
                        ps = ppool.tile([128, psum_cols], dt32)
                        for j in range(mm_per_ps):
                            n0 = pc * psum_cols + j * 512
                            for kc in range(n_kchunk):
                                k0, k1 = kc * 128, min(R, (kc + 1) * 128)
                                nc.tensor.matmul(
                                    ps[:, j * 512:(j + 1) * 512],
                                    lhsT_sb[k0:k1, m * 128:(m + 1) * 128],
                                    rhs_sb[k0:k1, n0:n0 + 512],
                                    start=(kc == 0),
                                    stop=(kc == n_kchunk - 1),
                                )
                        dst = ot[:, pc * psum_cols:(pc + 1) * psum_cols]
                        if copy_i % 4 < act_share:
                            nc.scalar.copy(dst, ps[:])
                        else:
                            nc.vector.tensor_copy(dst, ps[:])
                        copy_i += 1
                        if out_split > 1:
                            # stream this stripe to HBM as soon as it's copied
                            per = n_pschunk // out_split
                            if (pc + 1) % per == 0:
                                c0 = (pc + 1 - per) * psum_cols
                                c1 = (pc + 1) * psum_cols
                                nc.sync.dma_start(
                                    out[m * 128:(m + 1) * 128, c0:c1],
                                    ot[:, c0:c1])
                    if out_split == 1:
                        nc.sync.dma_start(out[m * 128:(m + 1) * 128, :], ot[:])
    nc.compile()
    return nc


def _get_nc(R):
    """Build (and cache) the production Bass module for contraction rank R."""
    if R in _COMPILED:
        return _COMPILED[R]
    nc = _build_nc(R, **_CFG)
    _COMPILED[R] = nc
    return nc


def _pack_split3(A, B):
    """f64 factors (R, 4096) -> bf16 K-stacked factors (3R, 4096)."""
    import ml_dtypes
    bf = ml_dtypes.bfloat16
    Ah = A.astype(bf)
    Al = (A - Ah.astype(np.float64)).astype(bf)
    Bh = B.astype(bf)
    Bl = (B - Bh.astype(np.float64)).astype(bf)
    Ap = np.concatenate([Ah, Al, Ah], axis=0)
    Bp = np.concatenate([Bh, Bh, Bl], axis=0)
    return Ap, Bp


def _make_in_maps(A, B):
    """A: (K, 4096) left factors, B: (K, 4096) right factors (any np dtype)."""
    rhs_np = np.ascontiguousarray(B)
    in_maps = []
    for c in range(_N_CORES):
        in_maps.append({
            "lhsT": np.ascontiguousarray(
                A[:, c * _ROWS_PER_CORE:(c + 1) * _ROWS_PER_CORE]
            ),
            "rhs": rhs_np,
        })
    return in_maps


def _run_device(A, B, trace=False):
    """A, B: (R, 4096) float64 factors.  Returns (psi_flat f32, results)."""
    from concourse.bass_utils import run_bass_kernel_spmd

    if _SPLIT3:
        Ap, Bp = _pack_split3(A, B)
    else:
        Ap, Bp = A.astype(np.float32), B.astype(np.float32)
    nc = _get_nc(Ap.shape[0])
    in_maps = _make_in_maps(Ap, Bp)
    res = run_bass_kernel_spmd(
        nc, in_maps, core_ids=list(range(_N_CORES)), trace=trace
    )
    flat = np.concatenate([r["out"].reshape(-1) for r in res.results])
    return flat, res


def kernel(states, gates, gate_wires):
    fact = _build_factors(states, gates, gate_wires)
    if fact is None:
        return _dense_fallback(states, gates, gate_wires)
    A, B = fact
    flat, _ = _run_device(A, B)
    return flat.reshape((2,) * _N_WIRES)
